# revision 10
# baseline (speedup 1.0000x reference)
"""Negative-sampling loss kernel for Trainium2 (8 NeuronCores, SPMD).

Strategy
--------
Data-parallel over batch B=262144 across 8 cores (32768 items each).
The whole computation is two scalar losses derived from per-item dot
products between gathered embedding rows:

    pos:  s_b = input_emb[t_b] . output_emb[c_b]          (B scores)
    neg:  s_bj = input_emb[t_b] . output_emb[n_bj]        (B*5 scores)

Each (t, other) pair is gathered independently on-device with the
custom `dma_gather` instruction (256B bf16 rows).  `dma_gather` uses
int16 indices, so each pair is bucketed by the 32768-row vocab windows
of both its words; every gather instruction then has a static window
base and in-window (<32768) indices.  Positive and negative pairs
share the same (t_win, o_win) tile groups, and each group's tail tile
is sized to a 128-row granule (instead of a full 1024-row chunk), which
cuts padded gather rows from ~7% to ~1.5%.  Gathers round-robin the 4
SWDGE queues at single-gather granularity so one queue's descriptor-
ring drain overlaps the next queue's prep.  Scores are computed on the
vector engine (bf16 multiply + segmented reduce) and DMA'd back; the
final softplus + mean runs on the host in float64 (1.5M scalars).

Tables are converted to bf16 on the host: score rel-error ~1% per item
averages out over 262144 items (final loss rel-error ~1e-7).
"""

from contextlib import ExitStack

import numpy as np
import ml_dtypes

import concourse.bass as bass
import concourse.bacc as bacc
import concourse.mybir as mybir
import concourse.tile as tile
from concourse.bass_utils import run_bass_kernel_spmd

VOCAB = 100000
D = 128
B = 262144
NEG = 5
NCORES = 8
BPC = B // NCORES          # 32768 items per core
WIN = 32768                # dma_gather int16 index window
NWIN = (VOCAB + WIN - 1) // WIN   # 4
CHUNK = 1024               # max gathered rows per dma_gather (HW ring cap)
GRAN = 128                 # tail tile granularity
NQ = 4                     # SWDGE queues

_cache = {}


def _group_tiles(cnt):
    """Chunk sizes covering cnt rows: CHUNK blocks + one 128-granule tail."""
    out = []
    while cnt > CHUNK:
        out.append(CHUNK)
        cnt -= CHUNK
    if cnt > 0:
        out.append(int(np.ceil(cnt / GRAN)) * GRAN)
    return out


def _build_nc(tile_meta):
    """One SPMD program: tile_meta[i] = (t_win, p_win, chunk)."""
    nc = bacc.Bacc(
        "TRN2",
        target_bir_lowering=False,
        debug=False,
        enable_asserts=False,
        num_swdge_queues=NQ,
    )
    total_rows = sum(ch for _, _, ch in tile_meta)
    nidx = total_rows // 16
    nscore = total_rows // 128
    in_emb = nc.dram_tensor("in_emb", [VOCAB, D], mybir.dt.bfloat16, kind="ExternalInput")
    out_emb = nc.dram_tensor("out_emb", [VOCAB, D], mybir.dt.bfloat16, kind="ExternalInput")
    # concatenated per-tile int16 indices, 16-wrapped, replicated to 128 partitions
    cidx = nc.dram_tensor("cidx", [128, 2 * nidx], mybir.dt.int16, kind="ExternalInput")
    sc_out = nc.dram_tensor("sc_out", [128, nscore], mybir.dt.float32, kind="ExternalOutput")

    with tile.TileContext(nc) as tc, ExitStack() as ctx:
        idxp = ctx.enter_context(tc.tile_pool(name="idx", bufs=12))
        gatp = ctx.enter_context(tc.tile_pool(name="gat", bufs=8))
        scp = ctx.enter_context(tc.tile_pool(name="sc", bufs=1))

        sc_all = scp.tile([128, nscore], mybir.dt.float32)
        ioff = 0
        soff = 0
        for t, (t_win, p_win, ch) in enumerate(tile_meta):
            kpt = ch // 128
            ci = idxp.tile([128, 2 * (CHUNK // 16)], mybir.dt.int16, tag="ci")
            nc.sync.dma_start(out=ci[:, :2 * (ch // 16)],
                              in_=cidx[:, 2 * ioff:2 * ioff + 2 * (ch // 16)])

            tt = gatp.tile([128, (CHUNK // 128) * D], mybir.dt.bfloat16, tag="tt")
            pt = gatp.tile([128, (CHUNK // 128) * D], mybir.dt.bfloat16, tag="pt")
            tt3 = tt[:, :kpt * D].rearrange("p (g d) -> p g d", d=D)
            pt3 = pt[:, :kpt * D].rearrange("p (g d) -> p g d", d=D)
            nc.gpsimd.dma_gather(
                tt3,
                in_emb[t_win * WIN:, :],
                ci[:, :ch // 16], ch, ch, D, elem_step=D,
                queue_num=(2 * t) % NQ,
            )
            nc.gpsimd.dma_gather(
                pt3,
                out_emb[p_win * WIN:, :],
                ci[:, ch // 16:2 * (ch // 16)], ch, ch, D, elem_step=D,
                queue_num=(2 * t + 1) % NQ,
            )
            nc.vector.tensor_tensor(out=pt3, in0=pt3, in1=tt3, op=mybir.AluOpType.mult)
            nc.vector.tensor_reduce(
                out=sc_all[:, soff:soff + kpt],
                in_=pt3, axis=mybir.AxisListType.X, op=mybir.AluOpType.add)
            ioff += ch // 16
            soff += kpt
        nc.sync.dma_start(out=sc_out[:], in_=sc_all[:])
    nc.finalize()
    return nc


def _plan_and_pack(target_words, context_words, negative_words):
    """Bucket all (t, other) pairs per core by (t_win, o_win) with pos and
    neg pairs sharing groups; build the shared tile schedule and per-core
    index tensors."""
    t_w = np.asarray(target_words).astype(np.int64).reshape(NCORES, BPC)
    c_w = np.asarray(context_words).astype(np.int64).reshape(NCORES, BPC)
    n_w = np.asarray(negative_words).astype(np.int64).reshape(NCORES, BPC, NEG)

    # per core: 6*BPC pairs (t, other); first BPC are positive (context)
    NP = BPC * (1 + NEG)
    t_all = np.concatenate([t_w, np.repeat(t_w, NEG, axis=1)], axis=1)   # [NCORES, NP]
    o_all = np.concatenate([c_w, n_w.reshape(NCORES, -1)], axis=1)       # [NCORES, NP]
    is_pos_pair = np.zeros(NP, bool)
    is_pos_pair[:BPC] = True

    key = (t_all // WIN) * NWIN + (o_all // WIN)   # [NCORES, NP]

    NG = NWIN * NWIN
    cnt = np.zeros((NCORES, NG), np.int64)
    for c in range(NCORES):
        cnt[c] = np.bincount(key[c], minlength=NG)

    tile_meta = []
    g_rows = []          # row offset of each group's slot range
    g_cap = []
    row_off = 0
    for g in range(NG):
        g_rows.append(row_off)
        chs = _group_tiles(int(cnt[:, g].max()))
        for ch in chs:
            tile_meta.append((g // NWIN, g % NWIN, ch))
            row_off += ch
        g_cap.append(sum(chs))
    total_rows = row_off

    tidx_all = np.zeros((NCORES, total_rows), np.int16)
    pidx_all = np.zeros((NCORES, total_rows), np.int16)
    valid_all = np.zeros((NCORES, total_rows), bool)
    ispos_all = np.zeros((NCORES, total_rows), bool)
    for c in range(NCORES):
        order = np.argsort(key[c], kind="stable")
        ks = key[c][order]
        tw = t_all[c][order]
        ow = o_all[c][order]
        ip = is_pos_pair[order]
        for g in range(NG):
            lo = np.searchsorted(ks, g, "left")
            hi = np.searchsorted(ks, g, "right")
            n = hi - lo
            if n == 0:
                continue
            assert n <= g_cap[g]
            s0 = g_rows[g]
            tidx_all[c, s0:s0 + n] = (tw[lo:hi] - (g // NWIN) * WIN).astype(np.int16)
            pidx_all[c, s0:s0 + n] = (ow[lo:hi] - (g % NWIN) * WIN).astype(np.int16)
            valid_all[c, s0:s0 + n] = True
            ispos_all[c, s0:s0 + n] = ip[lo:hi]

    assert (tidx_all >= 0).all() and (pidx_all >= 0).all()

    def pack_cidx(tv, pv):
        """Per-tile 16-wrap of t-idx and p-idx into one [128, 2*nidx] tensor:
        per tile, [t-block | p-block]; value i -> channel i%16, slot i//16;
        replicated 8x across the 128 partitions."""
        cols = []
        off = 0
        for _, _, ch in tile_meta:
            for v in (tv, pv):
                w = v[off:off + ch].reshape(ch // 16, 16).T  # [16, ch//16]
                cols.append(np.tile(w, (8, 1)))              # [128, ch//16]
            off += ch
        return np.concatenate(cols, axis=1).copy()

    per_core = []
    for c in range(NCORES):
        per_core.append({"cidx": pack_cidx(tidx_all[c], pidx_all[c])})
    return tile_meta, per_core, valid_all, ispos_all


def _unpack(sc_outs, tile_meta, valid_all, ispos_all):
    """sc_outs: per-core [128, total_rows//128] float32 score tensors.
    Within tile t, slot s lives at sc[s % 128, soff + s//128]."""
    pos_sum = 0.0
    neg_sum = 0.0
    for c in range(len(sc_outs)):
        sc = np.asarray(sc_outs[c]).astype(np.float64)
        parts = []
        soff = 0
        for (_, _, ch) in tile_meta:
            kpt = ch // 128
            parts.append(sc[:, soff:soff + kpt].T.reshape(-1))
            soff += kpt
        flat = np.concatenate(parts)
        v = valid_all[c]
        ip = ispos_all[c]
        pos_scores = flat[v & ip]
        neg_scores = flat[v & ~ip]
        pos_sum += np.logaddexp(0.0, -pos_scores).sum()
        neg_sum += np.logaddexp(0.0, neg_scores).sum()
    positive_loss = np.float32(pos_sum / B)
    negative_loss = np.float32(neg_sum / (B * NEG))
    return positive_loss, negative_loss


def kernel(target_words, context_words, negative_words, input_emb, output_emb,
           _want_results=False, _trace=False):
    input_emb = np.asarray(input_emb)
    output_emb = np.asarray(output_emb)
    in_bf = input_emb.astype(ml_dtypes.bfloat16)
    out_bf = output_emb.astype(ml_dtypes.bfloat16)

    tile_meta, per_core, valid_all, ispos_all = _plan_and_pack(
        target_words, context_words, negative_words)

    key = tuple(tile_meta)
    if key not in _cache:
        _cache[key] = _build_nc(tile_meta)
    nc = _cache[key]

    in_maps = []
    for c in range(NCORES):
        in_maps.append({
            "in_emb": np.asarray(in_bf),
            "out_emb": np.asarray(out_bf),
            "cidx": per_core[c]["cidx"],
        })
    br = run_bass_kernel_spmd(nc, in_maps, core_ids=list(range(NCORES)),
                              trace=_trace)

    positive_loss, negative_loss = _unpack(
        [br.results[c]["sc_out"] for c in range(NCORES)],
        tile_meta, valid_all, ispos_all)
    if _want_results:
        return (positive_loss, negative_loss), br
    return (positive_loss, negative_loss)


# revision 11
# speedup vs baseline: 1.0121x; 1.0121x over previous
"""Negative-sampling loss kernel for Trainium2 (8 NeuronCores, SPMD).

Strategy
--------
Data-parallel over batch B=262144 across 8 cores (32768 items each).
The whole computation is two scalar losses derived from per-item dot
products between gathered embedding rows:

    pos:  s_b = input_emb[t_b] . output_emb[c_b]          (B scores)
    neg:  s_bj = input_emb[t_b] . output_emb[n_bj]        (B*5 scores)

Each (t, other) pair is gathered independently on-device with the
custom `dma_gather` instruction (256B bf16 rows).  `dma_gather` uses
int16 indices, so each pair is bucketed by the 32768-row vocab windows
of both its words; every gather instruction then has a static window
base and in-window (<32768) indices.  Positive and negative pairs
share the same (t_win, o_win) tile groups, and each group's tail tile
is sized to a 128-row granule (instead of a full 1024-row chunk), which
cuts padded gather rows from ~7% to ~1.5%.  Gathers round-robin the 4
SWDGE queues at single-gather granularity so one queue's descriptor-
ring drain overlaps the next queue's prep.  Scores are computed on the
vector engine (bf16 multiply + segmented reduce) and DMA'd back; the
final softplus + mean runs on the host in float64 (1.5M scalars).

Tables are converted to bf16 on the host: score rel-error ~1% per item
averages out over 262144 items (final loss rel-error ~1e-7).
"""

from contextlib import ExitStack

import numpy as np
import ml_dtypes

import concourse.bass as bass
import concourse.bacc as bacc
import concourse.mybir as mybir
import concourse.tile as tile
from concourse.bass_utils import run_bass_kernel_spmd

VOCAB = 100000
D = 128
B = 262144
NEG = 5
NCORES = 8
BPC = B // NCORES          # 32768 items per core
WIN = 32768                # dma_gather int16 index window
NWIN = (VOCAB + WIN - 1) // WIN   # 4
CHUNK = 1024               # max gathered rows per dma_gather (HW ring cap)
GRAN = 128                 # tail tile granularity
NQ = 4                     # SWDGE queues

_cache = {}


def _group_tiles(cnt):
    """Chunk sizes covering cnt rows: CHUNK blocks + one 128-granule tail."""
    out = []
    while cnt > CHUNK:
        out.append(CHUNK)
        cnt -= CHUNK
    if cnt > 0:
        out.append(int(np.ceil(cnt / GRAN)) * GRAN)
    return out


def _build_nc(tile_meta):
    """One SPMD program: tile_meta[i] = (t_win, p_win, chunk)."""
    nc = bacc.Bacc(
        "TRN2",
        target_bir_lowering=False,
        debug=False,
        enable_asserts=False,
        num_swdge_queues=NQ,
    )
    total_rows = sum(ch for _, _, ch in tile_meta)
    nidx = total_rows // 16
    nscore = total_rows // 128
    in_emb = nc.dram_tensor("in_emb", [VOCAB, D], mybir.dt.bfloat16, kind="ExternalInput")
    out_emb = nc.dram_tensor("out_emb", [VOCAB, D], mybir.dt.bfloat16, kind="ExternalInput")
    # concatenated per-tile int16 indices, 16-wrapped, replicated to 128 partitions
    tidx = nc.dram_tensor("tidx", [128, nidx], mybir.dt.int16, kind="ExternalInput")
    pidx = nc.dram_tensor("pidx", [128, nidx], mybir.dt.int16, kind="ExternalInput")
    sc_out = nc.dram_tensor("sc_out", [128, nscore], mybir.dt.float32, kind="ExternalOutput")

    with tile.TileContext(nc) as tc, ExitStack() as ctx:
        idxp = ctx.enter_context(tc.tile_pool(name="idx", bufs=12))
        gatp = ctx.enter_context(tc.tile_pool(name="gat", bufs=8))
        scp = ctx.enter_context(tc.tile_pool(name="sc", bufs=1))

        sc_all = scp.tile([128, nscore], mybir.dt.float32)
        ioff = 0
        soff = 0
        for t, (t_win, p_win, ch) in enumerate(tile_meta):
            kpt = ch // 128
            ti = idxp.tile([128, CHUNK // 16], mybir.dt.int16, tag="ti")
            pi = idxp.tile([128, CHUNK // 16], mybir.dt.int16, tag="pi")
            nc.sync.dma_start(out=ti[:, :ch // 16], in_=tidx[:, ioff:ioff + ch // 16])
            nc.sync.dma_start(out=pi[:, :ch // 16], in_=pidx[:, ioff:ioff + ch // 16])

            tt = gatp.tile([128, (CHUNK // 128) * D], mybir.dt.bfloat16, tag="tt")
            pt = gatp.tile([128, (CHUNK // 128) * D], mybir.dt.bfloat16, tag="pt")
            tt3 = tt[:, :kpt * D].rearrange("p (g d) -> p g d", d=D)
            pt3 = pt[:, :kpt * D].rearrange("p (g d) -> p g d", d=D)
            nc.gpsimd.dma_gather(
                tt3,
                in_emb[t_win * WIN:, :],
                ti[:, :ch // 16], ch, ch, D, elem_step=D,
                queue_num=(2 * t) % NQ,
            )
            nc.gpsimd.dma_gather(
                pt3,
                out_emb[p_win * WIN:, :],
                pi[:, :ch // 16], ch, ch, D, elem_step=D,
                queue_num=(2 * t + 1) % NQ,
            )
            nc.vector.tensor_tensor(out=pt3, in0=pt3, in1=tt3, op=mybir.AluOpType.mult)
            nc.vector.tensor_reduce(
                out=sc_all[:, soff:soff + kpt],
                in_=pt3, axis=mybir.AxisListType.X, op=mybir.AluOpType.add)
            ioff += ch // 16
            soff += kpt
        nc.sync.dma_start(out=sc_out[:], in_=sc_all[:])
    nc.finalize()
    return nc


def _plan_and_pack(target_words, context_words, negative_words):
    """Bucket all (t, other) pairs per core by (t_win, o_win) with pos and
    neg pairs sharing groups; build the shared tile schedule and per-core
    index tensors."""
    t_w = np.asarray(target_words).astype(np.int64).reshape(NCORES, BPC)
    c_w = np.asarray(context_words).astype(np.int64).reshape(NCORES, BPC)
    n_w = np.asarray(negative_words).astype(np.int64).reshape(NCORES, BPC, NEG)

    # per core: 6*BPC pairs (t, other); first BPC are positive (context)
    NP = BPC * (1 + NEG)
    t_all = np.concatenate([t_w, np.repeat(t_w, NEG, axis=1)], axis=1)   # [NCORES, NP]
    o_all = np.concatenate([c_w, n_w.reshape(NCORES, -1)], axis=1)       # [NCORES, NP]
    is_pos_pair = np.zeros(NP, bool)
    is_pos_pair[:BPC] = True

    key = (t_all // WIN) * NWIN + (o_all // WIN)   # [NCORES, NP]

    NG = NWIN * NWIN
    cnt = np.zeros((NCORES, NG), np.int64)
    for c in range(NCORES):
        cnt[c] = np.bincount(key[c], minlength=NG)

    tile_meta = []
    g_rows = []          # row offset of each group's slot range
    g_cap = []
    row_off = 0
    for g in range(NG):
        g_rows.append(row_off)
        chs = _group_tiles(int(cnt[:, g].max()))
        for ch in chs:
            tile_meta.append((g // NWIN, g % NWIN, ch))
            row_off += ch
        g_cap.append(sum(chs))
    total_rows = row_off

    tidx_all = np.zeros((NCORES, total_rows), np.int16)
    pidx_all = np.zeros((NCORES, total_rows), np.int16)
    valid_all = np.zeros((NCORES, total_rows), bool)
    ispos_all = np.zeros((NCORES, total_rows), bool)
    for c in range(NCORES):
        order = np.argsort(key[c], kind="stable")
        ks = key[c][order]
        tw = t_all[c][order]
        ow = o_all[c][order]
        ip = is_pos_pair[order]
        for g in range(NG):
            lo = np.searchsorted(ks, g, "left")
            hi = np.searchsorted(ks, g, "right")
            n = hi - lo
            if n == 0:
                continue
            assert n <= g_cap[g]
            s0 = g_rows[g]
            tidx_all[c, s0:s0 + n] = (tw[lo:hi] - (g // NWIN) * WIN).astype(np.int16)
            pidx_all[c, s0:s0 + n] = (ow[lo:hi] - (g % NWIN) * WIN).astype(np.int16)
            valid_all[c, s0:s0 + n] = True
            ispos_all[c, s0:s0 + n] = ip[lo:hi]

    assert (tidx_all >= 0).all() and (pidx_all >= 0).all()

    def wrap16(v):
        """Per-tile 16-wrap: [total_rows] -> [128, total_rows//16].
        Within each tile, value i -> channel i%16, slot i//16; replicated
        8x across the 128 partitions."""
        cols = []
        off = 0
        for _, _, ch in tile_meta:
            w = v[off:off + ch].reshape(ch // 16, 16).T     # [16, ch//16]
            cols.append(np.tile(w, (8, 1)))                 # [128, ch//16]
            off += ch
        return np.concatenate(cols, axis=1).copy()

    per_core = []
    for c in range(NCORES):
        per_core.append({
            "tidx": wrap16(tidx_all[c]),
            "pidx": wrap16(pidx_all[c]),
        })
    return tile_meta, per_core, valid_all, ispos_all


def _unpack(sc_outs, tile_meta, valid_all, ispos_all):
    """sc_outs: per-core [128, total_rows//128] float32 score tensors.
    Within tile t, slot s lives at sc[s % 128, soff + s//128]."""
    pos_sum = 0.0
    neg_sum = 0.0
    for c in range(len(sc_outs)):
        sc = np.asarray(sc_outs[c]).astype(np.float64)
        parts = []
        soff = 0
        for (_, _, ch) in tile_meta:
            kpt = ch // 128
            parts.append(sc[:, soff:soff + kpt].T.reshape(-1))
            soff += kpt
        flat = np.concatenate(parts)
        v = valid_all[c]
        ip = ispos_all[c]
        pos_scores = flat[v & ip]
        neg_scores = flat[v & ~ip]
        pos_sum += np.logaddexp(0.0, -pos_scores).sum()
        neg_sum += np.logaddexp(0.0, neg_scores).sum()
    positive_loss = np.float32(pos_sum / B)
    negative_loss = np.float32(neg_sum / (B * NEG))
    return positive_loss, negative_loss


def kernel(target_words, context_words, negative_words, input_emb, output_emb,
           _want_results=False, _trace=False):
    input_emb = np.asarray(input_emb)
    output_emb = np.asarray(output_emb)
    in_bf = input_emb.astype(ml_dtypes.bfloat16)
    out_bf = output_emb.astype(ml_dtypes.bfloat16)

    tile_meta, per_core, valid_all, ispos_all = _plan_and_pack(
        target_words, context_words, negative_words)

    key = tuple(tile_meta)
    if key not in _cache:
        _cache[key] = _build_nc(tile_meta)
    nc = _cache[key]

    in_maps = []
    for c in range(NCORES):
        in_maps.append({
            "in_emb": np.asarray(in_bf),
            "out_emb": np.asarray(out_bf),
            "tidx": per_core[c]["tidx"],
            "pidx": per_core[c]["pidx"],
        })
    br = run_bass_kernel_spmd(nc, in_maps, core_ids=list(range(NCORES)),
                              trace=_trace)

    positive_loss, negative_loss = _unpack(
        [br.results[c]["sc_out"] for c in range(NCORES)],
        tile_meta, valid_all, ispos_all)
    if _want_results:
        return (positive_loss, negative_loss), br
    return (positive_loss, negative_loss)
